# revision 13
# baseline (speedup 1.0000x reference)
"""Trainium2 Bass kernel for nn_DigitConvolutionalModel (dense_cnn).

Math: the 3x3 valid conv is linear in x, so it folds into fc1:
    conv(x) @ fc1_w.T == x @ (C @ fc1_w.T)  with C [784, 676] the conv matrix.
The whole model is then a 3-layer MLP:
    out = relu(relu(x @ W1 + b1) @ W2 + b2) @ W3 + b3
with W1 = C @ fc1_w.T [784,512], W2 = fc2_w.T [512,512], W3 = out_w.T [512,10].

Sharding: pure data parallelism; batch 32768 -> 8 cores x 4096 rows.

On-chip formulation is fully transposed (features on SBUF partitions, batch on
the free dim): each layer computes h^T = act(W_l as lhsT, rhs = h_{l-1}^T).

PE-array scheduling (HW-measured model):
 - flags!=3 (accumulating) matmuls drain PSUM at ~259ns/512 cols; flags=3
   (start+stop) at ~216ns; LDWEIGHTS hides in the drain windows.
 - Layer 1 contraction (K=784): features 0..511 run in fp8-E4M3 DoubleRow
   (K=256/pass -> 2 passes instead of 4; measured rel err 1.81% < 2e-2).
   x scaled by 16, W1 by 128 (power-of-2, exact); the h1 activation rescales
   by 1/2048.  Features 512..767 (bf16) run FIRST so the wide DoubleRow
   LDWEIGHTS (225ns) hide behind bf16 drains.  The 16-row remainder is ONE
   pass of 4 concurrent row-tiled matmuls (tile_position=(32*mi,0), K=16).
 - Layer 3 (M=10): ONE concurrent pass of 4 col-tiled flags=3 matmuls
   (tile_position=(0,32*ki)) writes the k-partials to partition strips of a
   PSUM bank; a full-partition DVE scalar_tensor_tensor copy (bf16) + one
   matmul against a 0/1 selection matrix reduces the strips.  Two banks
   alternate by chunk parity so pack(n-2) and reduce(n-3) sit adjacent in
   the small-shape window and layer 2 runs uninterrupted.
   (tensor_copy would lower to CAST and pull an extra ucode library load
   into the preamble, +3us; stt avoids that.)
 - h2 relu+bias runs on the idle vector engine.
 - Warm-up: dummy N=128 matmuls + bridges keep the PE busy through the
   preamble/DMA fill so HAM reaches K=8/8 early.
"""

import numpy as np
import ml_dtypes

NCORES = 8
B = 32768
BC = B // NCORES  # rows per core
CH = 512          # batch chunk = matmul moving free dim = one fp32 PSUM bank
NCH = BC // CH
MT1 = 4           # 512 out feats = 4 m-tiles of 128
KT2, MT2 = 4, 4   # layer-2: K=512, M=512
KT3, MO = 4, 10   # layer-3: K=512, M=10
XW16 = 3 * CH     # bf16 x cols: k4 | k5 | replicated k6 block
SX = 16.0         # fp8 x scale (pow2, exact)
SW = 128.0        # fp8 W1 scale
SCALE = SX * SW
N_WARM = 36       # N=128 dummy matmuls bridging preamble -> first k-tile

_cache = {}


def _build():
    """Trace + compile the Bass program once per process."""
    if "nc" in _cache:
        return _cache["nc"]

    from contextlib import ExitStack

    import concourse.bass as bass
    import concourse.tile as tile
    from concourse import bacc, mybir
    from concourse.bass import ts, ds

    DT = mybir.dt.bfloat16
    F8 = mybir.dt.float8e4
    F32 = mybir.dt.float32
    DR = mybir.MatmulPerfMode.DoubleRow
    Relu = mybir.ActivationFunctionType.Relu
    Ident = mybir.ActivationFunctionType.Identity
    Add = mybir.AluOpType.add
    Max = mybir.AluOpType.max

    from concourse.vector_clock import ScopedClock

    class _FastExitTileContext(tile.TileContext):
        """Skip the exit semaphore-clear chain + second barrier (~2us tail)."""

        def _drain_and_barrier(self, tick_clock, wait_clock):
            drain_inst = self.nc.sync.drain()
            wait_clock.add_sem_waits(
                drain_inst.ins, ScopedClock({None: tick_clock.global_clock})
            )
            popped = self.nc._tile_sem_poison_stack.pop()
            assert popped is self._sem_poison

    nc = bacc.Bacc(
        "TRN2",
        target_bir_lowering=False,
        debug=False,
        enable_asserts=False,
        num_devices=NCORES,
        enable_partition_id=False,
    )

    # fp8 x: [chunk, partition, 2P+ko, col]; fp8 W1: [partition, 2P+ko, m]
    x8_d = nc.dram_tensor("x8", [NCH, 128, 4, CH], F8, kind="ExternalInput")
    x16_d = nc.dram_tensor("x16", [NCH, 128, XW16], DT, kind="ExternalInput")
    w18_d = nc.dram_tensor("w18", [128, 4, 512], F8, kind="ExternalInput")
    w1_d = nc.dram_tensor("w1", [128, 2 * 512], DT, kind="ExternalInput")
    w2_d = nc.dram_tensor("w2", [128, KT2 * 512], DT, kind="ExternalInput")
    # merged small consts, one DMA: w1 k6 block | w3 k-tiles | 0/1 sel matrix
    cst_d = nc.dram_tensor("cst", [128, 184], DT, kind="ExternalInput")
    b_d = nc.dram_tensor("b", [128, MT1 + MT2 + 1], F32, kind="ExternalInput")
    out_d = nc.dram_tensor("out", [MO, BC], F32, kind="ExternalOutput")

    with _FastExitTileContext(nc) as tc, ExitStack() as ctx:
        consts = ctx.enter_context(tc.tile_pool(name="consts", bufs=1))
        x8_pool = ctx.enter_context(tc.tile_pool(name="x8p", bufs=5))
        x16_pool = ctx.enter_context(tc.tile_pool(name="x16p", bufs=5))
        h1_pool = ctx.enter_context(tc.tile_pool(name="h1", bufs=8))
        h2_pool = ctx.enter_context(tc.tile_pool(name="h2", bufs=8))
        sb3_pool = ctx.enter_context(tc.tile_pool(name="sb3", bufs=2))
        oc_pool = ctx.enter_context(tc.tile_pool(name="oc", bufs=2))
        ps1 = ctx.enter_context(tc.tile_pool(name="ps1", bufs=4, space="PSUM"))
        ps2 = ctx.enter_context(tc.tile_pool(name="ps2", bufs=2, space="PSUM"))
        ps3 = ctx.enter_context(tc.tile_pool(name="ps3", bufs=2, space="PSUM"))

        # --- PE pre-warm from the earliest post-preamble instant ---
        warm_sb = consts.tile([128, 128], DT, name="warm_sb")
        nc.vector.memset(warm_sb[:], 0.0)
        zeros = consts.tile([128, CH], DT, name="zeros")
        nc.vector.memset(zeros[:], 0.0)
        warm_ps = ps2.tile([128, 128], F32, name="warm_ps", tag="ps2")
        for _ in range(N_WARM):
            nc.tensor.matmul(warm_ps[:], warm_sb[:], warm_sb[:],
                             start=True, stop=True)

        def bridge(k):
            for _ in range(k):
                nc.tensor.matmul(warm_ps[:], warm_sb[:], warm_sb[:],
                                 start=True, stop=True)

        # --- input DMAs ---
        # ring A = scalar engine queue, ring B = sync engine queue.
        x8_0 = consts.tile([128, 4, CH], F8, name="x8_0")
        nc.scalar.dma_start(x8_0[:, 0:2, :], x8_d[0][:, 0:2, :])
        w18_sb = consts.tile([128, 4, 512], F8, name="w18_sb")
        nc.sync.dma_start(w18_sb[:, 0:2, :], w18_d[:, 0:2, :])

        x16_0 = consts.tile([128, XW16], DT, name="x16_0")
        nc.scalar.dma_start(x16_0[:], x16_d[0])
        nc.sync.dma_start(x8_0[:, 2:4, :], x8_d[0][:, 2:4, :])
        nc.sync.dma_start(w18_sb[:, 2:4, :], w18_d[:, 2:4, :])

        b_sb = consts.tile([128, MT1 + MT2 + 1], F32, name="b_sb")
        nc.sync.dma_start(b_sb[:], b_d[:])
        w1_sb = consts.tile([128, 2 * 512], DT, name="w1_sb")
        nc.sync.dma_start(w1_sb[:], w1_d[:])

        x8_1 = consts.tile([128, 4, CH], F8, name="x8_1")
        nc.scalar.dma_start(x8_1[:], x8_d[1])

        cst = consts.tile([128, 184], DT, name="cst")
        nc.sync.dma_start(cst[:], cst_d[:])
        x16_1 = consts.tile([128, XW16], DT, name="x16_1")
        nc.sync.dma_start(x16_1[:], x16_d[1])

        w2_sb = consts.tile([128, KT2 * 512], DT, name="w2_sb")
        nc.scalar.dma_start(w2_sb[:], w2_d[:])

        x8_2 = consts.tile([128, 4, CH], F8, name="x8_2")
        nc.sync.dma_start(x8_2[:], x8_d[2])
        x16_2 = consts.tile([128, XW16], DT, name="x16_2")
        nc.scalar.dma_start(x16_2[:], x16_d[2])

        x8c = [x8_0, x8_1, x8_2] + [None] * (NCH - 3)
        x16c = [x16_0, x16_1, x16_2] + [None] * (NCH - 3)

        def fetch(n):
            t8 = x8_pool.tile([128, 4, CH], F8, name=f"x8c{n}", tag="x8")
            t16 = x16_pool.tile([128, XW16], DT, name=f"x16c{n}", tag="x16")
            if n % 2 == 1:
                nc.scalar.dma_start(t8[:], x8_d[n])
                nc.sync.dma_start(t16[:], x16_d[n])
            else:
                nc.sync.dma_start(t8[:], x8_d[n])
                nc.scalar.dma_start(t16[:], x16_d[n])
            x8c[n] = t8
            x16c[n] = t16

        # ---- pipeline stages ----
        h1s = {}   # n -> h1 tiles
        h2s = {}   # n -> h2 tiles
        pscs = {}  # n -> layer-3 psum tile
        sb3s = {}  # n -> bf16 strip-copy tile

        # two alternating layer-3 PSUM banks; strips at partitions 32k..+10.
        # memset once: stale NaN/Inf in the never-written rows would poison
        # the 0-weighted lanes of the selection reduce (0*NaN = NaN).
        ps3_t = [ps3.tile([128, CH], F32, name=f"ps3_{i}", tag="ps3")
                 for i in range(2)]
        nc.vector.memset(ps3_t[0][:], 0.0)
        nc.vector.memset(ps3_t[1][:], 0.0)

        def l1_dr(n, ps, P, start=False):
            for mi in range(MT1):
                nc.tensor.matmul(
                    ps[mi][:],
                    w18_sb[:, 2 * P:2 * P + 2, ds(mi * 128, 128)],
                    x8c[n][:, 2 * P:2 * P + 2, :],
                    start=start, stop=False, perf_mode=DR,
                )

        def l1_kbf(n, ps, kb, start=False):
            for mi in range(MT1):
                nc.tensor.matmul(
                    ps[mi][:],
                    w1_sb[:, ds(kb * 512 + mi * 128, 128)],
                    x16c[n][:, ts(kb, CH)],
                    start=start, stop=False,
                )

        def l1_k6(n, ps):
            # 16-row remainder: 4 concurrent row-tiled matmuls, one pass
            for mi in range(MT1):
                nc.tensor.matmul(
                    ps[mi][:],
                    cst[32 * mi:32 * mi + 16, 0:128],
                    x16c[n][32 * mi:32 * mi + 16, ds(2 * CH, CH)],
                    start=False, stop=True,
                    tile_position=(32 * mi, 0),
                )

        def l1_acts(n, ps):
            h1t = [
                h1_pool.tile([128, CH], DT, name=f"h1_{n}_{mi}", tag="h1")
                for mi in range(MT1)
            ]
            for mi in range(MT1):
                nc.scalar.activation(
                    h1t[mi][:], ps[mi][:], Relu, bias=b_sb[:, mi:mi + 1],
                    scale=1.0 / SCALE,
                )
            h1s[n] = h1t

        def l3_pack(n):
            # ONE concurrent pass: 4 col-tiled flags=3 matmuls write the 4
            # k-partials of chunk n to partition strips of bank n%2
            psc = ps3_t[n % 2]
            pscs[n] = psc
            h2t = h2s.pop(n)
            for ki in range(KT3):
                nc.tensor.matmul(
                    psc[32 * ki:32 * ki + MO, :],
                    cst[:, ds(128 + ki * MO, MO)], h2t[ki][:],
                    start=True, stop=True,
                    tile_position=(0, 32 * ki),
                )

        def l3_copy(n):
            # full-partition DVE stt copies the strips to bf16 SBUF; emitted
            # after layer2 so it never delays the h2 stt chain (ps2 has only
            # 2 rotating banks)
            sb = sb3_pool.tile([128, CH], DT, name=f"sb3_{n}", tag="sb3")
            nc.vector.scalar_tensor_tensor(
                sb[:], pscs[n][:], zeros[:, 0:1], zeros[:], Add, Add,
            )
            sb3s[n] = sb

        def l3_fin(n):
            # reduce the 4 strips with a 0/1 selection matmul (contracts
            # over partitions), then bias + store
            psc = pscs.pop(n)
            nc.tensor.matmul(psc[:MO, :], cst[:, 168:168 + MO],
                             sb3s.pop(n)[:], start=True, stop=True)
            oct_ = oc_pool.tile([MO, CH], F32, name=f"oc_{n}", tag="oc")
            nc.scalar.activation(
                oct_[:], psc[:MO, :], Ident,
                bias=b_sb[:MO, MT1 + MT2:MT1 + MT2 + 1],
            )
            nc.sync.dma_start(out_d[:, ts(n, CH)], oct_[:],
                              single_packet=True)

        def layer2(n):
            h1t = h1s.pop(n)
            h2t = [None] * MT2
            seq_ps = None
            if n == NCH - 1:
                # tail chunk: layer 3 as 4 sequential accumulating matmuls
                # riding between the L2 mi-groups as each h2 ACT lands -- no
                # pack/copy/reduce chain left on the critical tail.  ps1
                # banks are idle in the epilogue.
                seq_ps = ps1.tile([128, CH], F32, name="ps3_seq", tag="ps1")
                pscs[n] = seq_ps
            for mi in range(MT2):
                ps = ps2.tile([128, CH], F32, name=f"ps2_{n}_{mi}", tag="ps2")
                for ki in range(KT2):
                    nc.tensor.matmul(
                        ps[:],
                        w2_sb[:, ds(ki * 512 + mi * 128, 128)],
                        h1t[ki][:],
                        start=(ki == 0), stop=(ki == KT2 - 1),
                    )
                h2t[mi] = h2_pool.tile([128, CH], DT, name=f"h2_{n}_{mi}",
                                       tag="h2")
                if n == NCH - 1:
                    # relu on the (then idle) scalar engine, then the L3
                    # k-partial matmul for this mi immediately
                    nc.scalar.activation(
                        h2t[mi][:], ps[:], Relu,
                        bias=b_sb[:, MT1 + mi:MT1 + mi + 1],
                    )
                    nc.tensor.matmul(
                        seq_ps[:MO, :], cst[:, ds(128 + mi * MO, MO)],
                        h2t[mi][:],
                        start=(mi == 0), stop=(mi == MT2 - 1),
                    )
                else:
                    # relu+bias on the vector engine: max(ps + b, 0)
                    nc.vector.scalar_tensor_tensor(
                        h2t[mi][:], ps[:], b_sb[:, MT1 + mi:MT1 + mi + 1],
                        zeros[:], Add, Max,
                    )
            h2s[n] = h2t

        def l3seq_fin(n):
            psc = pscs.pop(n)
            oct_ = oc_pool.tile([MO, CH], F32, name=f"oc_{n}", tag="oc")
            nc.scalar.activation(
                oct_[:], psc[:MO, :], Ident,
                bias=b_sb[:MO, MT1 + MT2:MT1 + MT2 + 1],
            )
            nc.sync.dma_start(out_d[:, ts(n, CH)], oct_[:],
                              single_packet=True)

        # ---- software pipeline ----
        # iter n: layer1(n); chunk n-2's layer-3 pack and chunk n-3's reduce
        # sit adjacent in the small-shape window after the k6 pass; layer2
        # of chunk n-1 then runs uninterrupted.
        for n in range(NCH):
            if 1 <= n <= NCH - 3:
                fetch(n + 2)
            ps = [
                ps1.tile([128, CH], F32, name=f"ps1_{n}_{mi}", tag="ps1")
                for mi in range(MT1)
            ]
            if n == 0:
                # chunk 0 follows DMA-arrival order: fp8 pieces land first
                bridge(8)
                l1_dr(n, ps, 0, start=True)
                bridge(12)
                l1_dr(n, ps, 1)
                bridge(6)
                l1_kbf(n, ps, 0)
                bridge(4)
                l1_kbf(n, ps, 1)
                bridge(3)
            else:
                # bf16 first: the wide DoubleRow LDWEIGHTS then hide behind
                # bf16 drain windows
                l1_kbf(n, ps, 0, start=True)
                l1_kbf(n, ps, 1)
                if n == 1:
                    # chunk-1 fp8 x is still in flight on the DMA ring;
                    # keep HAM busy so it doesn't re-throttle
                    bridge(6)
                l1_dr(n, ps, 0)
                if n == 1:
                    bridge(4)
                l1_dr(n, ps, 1)
            l1_k6(n, ps)
            if n >= 2:
                l3_pack(n - 2)
            if n >= 3:
                l3_fin(n - 3)
            l1_acts(n, ps)
            if n == 1:
                # cover the first h1-activation latency before layer2(0)
                bridge(8)
            if n >= 1:
                layer2(n - 1)
            if n >= 2:
                l3_copy(n - 2)

        # epilogue: chunk-5/6 L3 work overlaps layer2(7); chunk 7 finishes
        # through the sequential path with no copy/reduce on the tail
        l3_pack(NCH - 2)
        l3_copy(NCH - 2)
        l3_fin(NCH - 3)
        layer2(NCH - 1)
        l3_fin(NCH - 2)
        l3seq_fin(NCH - 1)

    nc.compile()
    _cache["nc"] = nc
    return nc


def _prep_inputs(x, conv_w, fc1_w, fc1_b, fc2_w, fc2_b, out_w, out_b):
    dt = ml_dtypes.bfloat16
    f8 = ml_dtypes.float8_e4m3
    f32 = np.float32

    def q8(a):
        return np.clip(a, -192, 192).astype(f8)

    # Conv as a [784, 676] matrix (exact in fp64), folded into fc1.
    C = np.zeros((784, 676), dtype=np.float64)
    oy, ox = np.meshgrid(np.arange(26), np.arange(26), indexing="ij")
    cols = (oy * 26 + ox).ravel()
    for ky in range(3):
        for kx in range(3):
            rows = ((oy + ky) * 28 + (ox + kx)).ravel()
            np.add.at(C, (rows, cols), float(conv_w[ky, kx]))
    W1 = C @ fc1_w.T.astype(np.float64)  # [784, 512]

    # fp8 W1 (features 0..511), DoubleRow block layout [p, 2P+ko, m]
    w18 = np.zeros((128, 4, 512), dtype=f8)
    for kk in range(4):
        w18[:, kk, :] = q8(SW * W1[kk * 128:(kk + 1) * 128, :])
    # bf16 W1 (features 512..767), scaled to match
    w1 = np.zeros((128, 2 * 512), dtype=np.float64)
    for kb in range(2):
        w1[:, kb * 512:(kb + 1) * 512] = \
            SW * W1[512 + kb * 128:512 + (kb + 1) * 128, :]
    w1 = w1.astype(f32).astype(dt)

    w2 = np.ascontiguousarray(
        np.ascontiguousarray(fc2_w.T).reshape(KT2, 128, 512).transpose(1, 0, 2)
    ).reshape(128, KT2 * 512).astype(f32).astype(dt)
    b3col = np.zeros((128, 1), dtype=np.float64)
    b3col[:MO, 0] = out_b
    b = np.ascontiguousarray(
        np.concatenate(
            [fc1_b.reshape(MT1, 128).T, fc2_b.reshape(MT2, 128).T, b3col],
            axis=1,
        )
    ).astype(f32)
    # merged consts: scaled w1 k6 block | w3 k-tiles | 0/1 selection matrix
    cstm = np.zeros((128, 184), dtype=np.float64)
    for j in range(4):
        cstm[32 * j:32 * j + 16, 0:128] = \
            SW * W1[768:784, 128 * j:128 * (j + 1)]
    cstm[:, 128:168] = np.ascontiguousarray(out_w.T.astype(np.float64)).reshape(
        KT3, 128, MO).transpose(1, 0, 2).reshape(128, KT3 * MO)
    for j in range(MO):
        for q in range(KT3):
            cstm[32 * q + j, 168 + j] = 1.0
    cstm = cstm.astype(f32).astype(dt)

    in_maps = []
    for c in range(NCORES):
        xc = x[c * BC:(c + 1) * BC].T.astype(np.float64)  # [784, BC]
        x8 = np.zeros((NCH, 128, 4, CH), dtype=f8)
        x16 = np.zeros((NCH, 128, XW16), dtype=dt)
        for n in range(NCH):
            cn = slice(n * CH, (n + 1) * CH)
            for kk in range(4):
                x8[n, :, kk, :] = q8(SX * xc[kk * 128:(kk + 1) * 128, cn])
            for kb in range(2):
                x16[n, :, kb * CH:(kb + 1) * CH] = (
                    SX * xc[512 + kb * 128:512 + (kb + 1) * 128, cn]
                ).astype(f32)
            rep = (SX * xc[768:784, cn]).astype(f32)
            for j in range(4):
                x16[n, 32 * j:32 * j + 16, 2 * CH:] = rep
        in_maps.append(
            {"x8": x8, "x16": x16, "w18": w18, "w1": w1, "w2": w2,
             "cst": cstm, "b": b}
        )
    return in_maps


def kernel(x, conv_w, fc1_w, fc1_b, fc2_w, fc2_b, out_w, out_b, _results=None):
    from concourse.bass_utils import run_bass_kernel_spmd

    x, conv_w, fc1_w, fc1_b, fc2_w, fc2_b, out_w, out_b = (
        np.asarray(a)
        for a in (x, conv_w, fc1_w, fc1_b, fc2_w, fc2_b, out_w, out_b)
    )
    nc = _build()
    in_maps = _prep_inputs(x, conv_w, fc1_w, fc1_b, fc2_w, fc2_b, out_w, out_b)
    res = run_bass_kernel_spmd(nc, in_maps, core_ids=list(range(NCORES)))
    if _results is not None:
        _results.append(res)
    out = np.empty((B, 10), dtype=np.float32)
    for c in range(NCORES):
        out[c * BC:(c + 1) * BC, :] = res.results[c]["out"].T
    return out


# revision 14
# speedup vs baseline: 1.0230x; 1.0230x over previous
"""Trainium2 Bass kernel for nn_DigitConvolutionalModel (dense_cnn).

Math: the 3x3 valid conv is linear in x, so it folds into fc1:
    conv(x) @ fc1_w.T == x @ (C @ fc1_w.T)  with C [784, 676] the conv matrix.
The whole model is then a 3-layer MLP:
    out = relu(relu(x @ W1 + b1) @ W2 + b2) @ W3 + b3
with W1 = C @ fc1_w.T [784,512], W2 = fc2_w.T [512,512], W3 = out_w.T [512,10].

Sharding: pure data parallelism; batch 32768 -> 8 cores x 4096 rows.

On-chip formulation is fully transposed (features on SBUF partitions, batch on
the free dim): each layer computes h^T = act(W_l as lhsT, rhs = h_{l-1}^T).

PE-array scheduling (HW-measured model):
 - flags!=3 (accumulating) matmuls drain PSUM at ~259ns/512 cols; flags=3
   (start+stop) at ~216ns; LDWEIGHTS hides in the drain windows.
 - Layer 1 contraction (K=784): features 0..511 run in fp8-E4M3 DoubleRow
   (K=256/pass -> 2 passes instead of 4; measured rel err 1.81% < 2e-2).
   x scaled by 16, W1 by 128 (power-of-2, exact); the h1 activation rescales
   by 1/2048.  Features 512..767 (bf16) run FIRST so the wide DoubleRow
   LDWEIGHTS (225ns) hide behind bf16 drains.  The 16-row remainder is ONE
   pass of 4 concurrent row-tiled matmuls (tile_position=(32*mi,0), K=16).
 - Layer 3 (M=10): ONE concurrent pass of 4 col-tiled flags=3 matmuls
   (tile_position=(0,32*ki)) writes the k-partials to partition strips of a
   PSUM bank; a full-partition DVE scalar_tensor_tensor copy (bf16) + one
   matmul against a 0/1 selection matrix reduces the strips.  Two banks
   alternate by chunk parity so pack(n-2) and reduce(n-3) sit adjacent in
   the small-shape window and layer 2 runs uninterrupted.
   (tensor_copy would lower to CAST and pull an extra ucode library load
   into the preamble, +3us; stt avoids that.)
 - h2 relu+bias runs on the idle vector engine.
 - Warm-up: dummy N=128 matmuls + bridges keep the PE busy through the
   preamble/DMA fill so HAM reaches K=8/8 early.
"""

import numpy as np
import ml_dtypes

NCORES = 8
B = 32768
BC = B // NCORES  # rows per core
CH = 512          # batch chunk = matmul moving free dim = one fp32 PSUM bank
NCH = BC // CH
MT1 = 4           # 512 out feats = 4 m-tiles of 128
KT2, MT2 = 4, 4   # layer-2: K=512, M=512
KT3, MO = 4, 10   # layer-3: K=512, M=10
XW16 = 3 * CH     # bf16 x cols: k4 | k5 | replicated k6 block
SX = 16.0         # fp8 x scale (pow2, exact)
SW = 128.0        # fp8 W1 scale
SCALE = SX * SW
N_WARM = 36       # N=128 dummy matmuls bridging preamble -> first k-tile

_cache = {}


def _build():
    """Trace + compile the Bass program once per process."""
    if "nc" in _cache:
        return _cache["nc"]

    from contextlib import ExitStack

    import concourse.bass as bass
    import concourse.tile as tile
    from concourse import bacc, mybir
    from concourse.bass import ts, ds

    DT = mybir.dt.bfloat16
    F8 = mybir.dt.float8e4
    F32 = mybir.dt.float32
    DR = mybir.MatmulPerfMode.DoubleRow
    Relu = mybir.ActivationFunctionType.Relu
    Ident = mybir.ActivationFunctionType.Identity
    Add = mybir.AluOpType.add
    Max = mybir.AluOpType.max

    from concourse.vector_clock import ScopedClock

    class _FastExitTileContext(tile.TileContext):
        """Skip the exit semaphore-clear chain + second barrier (~2us tail)."""

        def _drain_and_barrier(self, tick_clock, wait_clock):
            drain_inst = self.nc.sync.drain()
            wait_clock.add_sem_waits(
                drain_inst.ins, ScopedClock({None: tick_clock.global_clock})
            )
            popped = self.nc._tile_sem_poison_stack.pop()
            assert popped is self._sem_poison

    nc = bacc.Bacc(
        "TRN2",
        target_bir_lowering=False,
        debug=False,
        enable_asserts=False,
        num_devices=NCORES,
        enable_partition_id=False,
    )

    # fp8 x: [chunk, partition, 2P+ko, col]; fp8 W1: [partition, 2P+ko, m]
    x8_d = nc.dram_tensor("x8", [NCH, 128, 4, CH], F8, kind="ExternalInput")
    x16_d = nc.dram_tensor("x16", [NCH, 128, XW16], DT, kind="ExternalInput")
    w18_d = nc.dram_tensor("w18", [128, 4, 512], F8, kind="ExternalInput")
    w1_d = nc.dram_tensor("w1", [128, 2 * 512], DT, kind="ExternalInput")
    w2_d = nc.dram_tensor("w2", [128, KT2 * 512], DT, kind="ExternalInput")
    # merged small consts, one DMA: w1 k6 block | w3 k-tiles | 0/1 sel matrix
    cst_d = nc.dram_tensor("cst", [128, 184], DT, kind="ExternalInput")
    b_d = nc.dram_tensor("b", [128, MT1 + MT2 + 1], F32, kind="ExternalInput")
    out_d = nc.dram_tensor("out", [MO, BC], F32, kind="ExternalOutput")

    with _FastExitTileContext(nc) as tc, ExitStack() as ctx:
        consts = ctx.enter_context(tc.tile_pool(name="consts", bufs=1))
        x8_pool = ctx.enter_context(tc.tile_pool(name="x8p", bufs=5))
        x16_pool = ctx.enter_context(tc.tile_pool(name="x16p", bufs=5))
        h1_pool = ctx.enter_context(tc.tile_pool(name="h1", bufs=8))
        h2_pool = ctx.enter_context(tc.tile_pool(name="h2", bufs=8))
        sb3_pool = ctx.enter_context(tc.tile_pool(name="sb3", bufs=2))
        oc_pool = ctx.enter_context(tc.tile_pool(name="oc", bufs=2))
        ps1 = ctx.enter_context(tc.tile_pool(name="ps1", bufs=4, space="PSUM"))
        ps2 = ctx.enter_context(tc.tile_pool(name="ps2", bufs=2, space="PSUM"))
        ps3 = ctx.enter_context(tc.tile_pool(name="ps3", bufs=2, space="PSUM"))

        # --- PE pre-warm from the earliest post-preamble instant ---
        warm_sb = consts.tile([128, 128], DT, name="warm_sb")
        nc.vector.memset(warm_sb[:], 0.0)
        zeros = consts.tile([128, CH], DT, name="zeros")
        nc.vector.memset(zeros[:], 0.0)
        warm_ps = ps2.tile([128, 128], F32, name="warm_ps", tag="ps2")
        for _ in range(N_WARM):
            nc.tensor.matmul(warm_ps[:], warm_sb[:], warm_sb[:],
                             start=True, stop=True)

        def bridge(k):
            for _ in range(k):
                nc.tensor.matmul(warm_ps[:], warm_sb[:], warm_sb[:],
                                 start=True, stop=True)

        # --- input DMAs ---
        # ring A = scalar engine queue, ring B = sync engine queue.
        x8_0 = consts.tile([128, 4, CH], F8, name="x8_0")
        nc.scalar.dma_start(x8_0[:, 0:2, :], x8_d[0][:, 0:2, :])
        w18_sb = consts.tile([128, 4, 512], F8, name="w18_sb")
        nc.sync.dma_start(w18_sb[:, 0:2, :], w18_d[:, 0:2, :])

        x16_0 = consts.tile([128, XW16], DT, name="x16_0")
        nc.scalar.dma_start(x16_0[:], x16_d[0])
        nc.sync.dma_start(x8_0[:, 2:4, :], x8_d[0][:, 2:4, :])
        nc.sync.dma_start(w18_sb[:, 2:4, :], w18_d[:, 2:4, :])

        b_sb = consts.tile([128, MT1 + MT2 + 1], F32, name="b_sb")
        nc.sync.dma_start(b_sb[:], b_d[:])
        w1_sb = consts.tile([128, 2 * 512], DT, name="w1_sb")
        nc.sync.dma_start(w1_sb[:], w1_d[:])

        x8_1 = consts.tile([128, 4, CH], F8, name="x8_1")
        nc.scalar.dma_start(x8_1[:], x8_d[1])

        cst = consts.tile([128, 184], DT, name="cst")
        nc.sync.dma_start(cst[:], cst_d[:])
        x16_1 = consts.tile([128, XW16], DT, name="x16_1")
        nc.sync.dma_start(x16_1[:], x16_d[1])

        w2_sb = consts.tile([128, KT2 * 512], DT, name="w2_sb")
        nc.scalar.dma_start(w2_sb[:], w2_d[:])

        x8_2 = consts.tile([128, 4, CH], F8, name="x8_2")
        nc.sync.dma_start(x8_2[:], x8_d[2])
        x16_2 = consts.tile([128, XW16], DT, name="x16_2")
        nc.scalar.dma_start(x16_2[:], x16_d[2])

        x8c = [x8_0, x8_1, x8_2] + [None] * (NCH - 3)
        x16c = [x16_0, x16_1, x16_2] + [None] * (NCH - 3)

        def fetch(n):
            t8 = x8_pool.tile([128, 4, CH], F8, name=f"x8c{n}", tag="x8")
            t16 = x16_pool.tile([128, XW16], DT, name=f"x16c{n}", tag="x16")
            if n % 2 == 1:
                nc.scalar.dma_start(t8[:], x8_d[n])
                nc.sync.dma_start(t16[:], x16_d[n])
            else:
                nc.sync.dma_start(t8[:], x8_d[n])
                nc.scalar.dma_start(t16[:], x16_d[n])
            x8c[n] = t8
            x16c[n] = t16

        # ---- pipeline stages ----
        h1s = {}   # n -> h1 tiles
        h2s = {}   # n -> h2 tiles
        pscs = {}  # n -> layer-3 psum tile
        sb3s = {}  # n -> bf16 strip-copy tile

        # two alternating layer-3 PSUM banks; strips at partitions 32k..+10.
        # memset once: stale NaN/Inf in the never-written rows would poison
        # the 0-weighted lanes of the selection reduce (0*NaN = NaN).
        ps3_t = [ps3.tile([128, CH], F32, name=f"ps3_{i}", tag="ps3")
                 for i in range(2)]
        nc.vector.memset(ps3_t[0][:], 0.0)
        nc.vector.memset(ps3_t[1][:], 0.0)

        def l1_dr(n, ps, P, start=False):
            for mi in range(MT1):
                nc.tensor.matmul(
                    ps[mi][:],
                    w18_sb[:, 2 * P:2 * P + 2, ds(mi * 128, 128)],
                    x8c[n][:, 2 * P:2 * P + 2, :],
                    start=start, stop=False, perf_mode=DR,
                )

        def l1_kbf(n, ps, kb, start=False):
            for mi in range(MT1):
                nc.tensor.matmul(
                    ps[mi][:],
                    w1_sb[:, ds(kb * 512 + mi * 128, 128)],
                    x16c[n][:, ts(kb, CH)],
                    start=start, stop=False,
                )

        def l1_k6(n, ps):
            # 16-row remainder: 4 concurrent row-tiled matmuls, one pass
            for mi in range(MT1):
                nc.tensor.matmul(
                    ps[mi][:],
                    cst[32 * mi:32 * mi + 16, 0:128],
                    x16c[n][32 * mi:32 * mi + 16, ds(2 * CH, CH)],
                    start=False, stop=True,
                    tile_position=(32 * mi, 0),
                )

        def l1_acts(n, ps):
            h1t = [
                h1_pool.tile([128, CH], DT, name=f"h1_{n}_{mi}", tag="h1")
                for mi in range(MT1)
            ]
            for mi in range(MT1):
                nc.scalar.activation(
                    h1t[mi][:], ps[mi][:], Relu, bias=b_sb[:, mi:mi + 1],
                    scale=1.0 / SCALE,
                )
            h1s[n] = h1t

        def l3_pack(n):
            # ONE concurrent pass: 4 col-tiled flags=3 matmuls write the 4
            # k-partials of chunk n to partition strips of bank n%2
            psc = ps3_t[n % 2]
            pscs[n] = psc
            h2t = h2s.pop(n)
            for ki in range(KT3):
                nc.tensor.matmul(
                    psc[32 * ki:32 * ki + MO, :],
                    cst[:, ds(128 + ki * MO, MO)], h2t[ki][:],
                    start=True, stop=True,
                    tile_position=(0, 32 * ki),
                )

        def l3_copy(n):
            # full-partition DVE stt copies the strips to bf16 SBUF; emitted
            # after layer2 so it never delays the h2 stt chain (ps2 has only
            # 2 rotating banks)
            sb = sb3_pool.tile([128, CH], DT, name=f"sb3_{n}", tag="sb3")
            nc.vector.scalar_tensor_tensor(
                sb[:], pscs[n][:], zeros[:, 0:1], zeros[:], Add, Add,
            )
            sb3s[n] = sb

        def l3_fin(n):
            # reduce the 4 strips with a 0/1 selection matmul (contracts
            # over partitions), then bias + store
            psc = pscs.pop(n)
            nc.tensor.matmul(psc[:MO, :], cst[:, 168:168 + MO],
                             sb3s.pop(n)[:], start=True, stop=True)
            oct_ = oc_pool.tile([MO, CH], F32, name=f"oc_{n}", tag="oc")
            nc.scalar.activation(
                oct_[:], psc[:MO, :], Ident,
                bias=b_sb[:MO, MT1 + MT2:MT1 + MT2 + 1],
            )
            nc.sync.dma_start(out_d[:, ts(n, CH)], oct_[:],
                              single_packet=True)

        def layer2(n):
            h1t = h1s.pop(n)
            h2t = [None] * MT2
            seq_ps = None
            if n == NCH - 1:
                # tail chunk: layer 3 as 4 sequential accumulating matmuls
                # riding between the L2 mi-groups as each h2 ACT lands -- no
                # pack/copy/reduce chain left on the critical tail.  ps1
                # banks are idle in the epilogue.
                seq_ps = ps1.tile([128, CH], F32, name="ps3_seq", tag="ps1")
                pscs[n] = seq_ps
            for mi in range(MT2):
                ps = ps2.tile([128, CH], F32, name=f"ps2_{n}_{mi}", tag="ps2")
                for ki in range(KT2):
                    nc.tensor.matmul(
                        ps[:],
                        w2_sb[:, ds(ki * 512 + mi * 128, 128)],
                        h1t[ki][:],
                        start=(ki == 0), stop=(ki == KT2 - 1),
                    )
                h2t[mi] = h2_pool.tile([128, CH], DT, name=f"h2_{n}_{mi}",
                                       tag="h2")
                if n == NCH - 1:
                    # relu on the (then idle) scalar engine, then the L3
                    # k-partial matmul for this mi immediately
                    nc.scalar.activation(
                        h2t[mi][:], ps[:], Relu,
                        bias=b_sb[:, MT1 + mi:MT1 + mi + 1],
                    )
                    nc.tensor.matmul(
                        seq_ps[:MO, :], cst[:, ds(128 + mi * MO, MO)],
                        h2t[mi][:],
                        start=(mi == 0), stop=(mi == MT2 - 1),
                    )
                else:
                    # relu+bias on the vector engine: max(ps + b, 0)
                    nc.vector.scalar_tensor_tensor(
                        h2t[mi][:], ps[:], b_sb[:, MT1 + mi:MT1 + mi + 1],
                        zeros[:], Add, Max,
                    )
            h2s[n] = h2t

        def l3seq_fin(n):
            # tail chunk: store via the scalar ring -- the sync engine is
            # busy issuing chunk-6's store and then runs the exit sequence;
            # splitting the two final DMAs across rings shortens the tail
            psc = pscs.pop(n)
            oct_ = oc_pool.tile([MO, CH], F32, name=f"oc_{n}", tag="oc")
            nc.scalar.activation(
                oct_[:], psc[:MO, :], Ident,
                bias=b_sb[:MO, MT1 + MT2:MT1 + MT2 + 1],
            )
            nc.scalar.dma_start(out_d[:, ts(n, CH)], oct_[:],
                                single_packet=True)

        # ---- software pipeline ----
        # iter n: layer1(n); chunk n-2's layer-3 pack and chunk n-3's reduce
        # sit adjacent in the small-shape window after the k6 pass; layer2
        # of chunk n-1 then runs uninterrupted.
        for n in range(NCH):
            if 1 <= n <= NCH - 3:
                fetch(n + 2)
            ps = [
                ps1.tile([128, CH], F32, name=f"ps1_{n}_{mi}", tag="ps1")
                for mi in range(MT1)
            ]
            if n == 0:
                # chunk 0 follows DMA-arrival order: fp8 pieces land first
                bridge(8)
                l1_dr(n, ps, 0, start=True)
                bridge(12)
                l1_dr(n, ps, 1)
                bridge(6)
                l1_kbf(n, ps, 0)
                bridge(4)
                l1_kbf(n, ps, 1)
                bridge(3)
            else:
                # bf16 first: the wide DoubleRow LDWEIGHTS then hide behind
                # bf16 drain windows
                l1_kbf(n, ps, 0, start=True)
                l1_kbf(n, ps, 1)
                if n == 1:
                    # chunk-1 fp8 x is still in flight on the DMA ring;
                    # keep HAM busy so it doesn't re-throttle
                    bridge(6)
                l1_dr(n, ps, 0)
                if n == 1:
                    bridge(4)
                l1_dr(n, ps, 1)
            l1_k6(n, ps)
            if n >= 2:
                l3_pack(n - 2)
            if n >= 3:
                l3_fin(n - 3)
            l1_acts(n, ps)
            if n == 1:
                # cover the first h1-activation latency before layer2(0)
                bridge(8)
            if n >= 1:
                layer2(n - 1)
            if n >= 2:
                l3_copy(n - 2)

        # epilogue: chunk-5/6 L3 work overlaps layer2(7); chunk 7 finishes
        # through the sequential path with no copy/reduce on the tail
        l3_pack(NCH - 2)
        l3_copy(NCH - 2)
        l3_fin(NCH - 3)
        layer2(NCH - 1)
        l3_fin(NCH - 2)
        l3seq_fin(NCH - 1)

    nc.compile()
    _cache["nc"] = nc
    return nc


def _prep_inputs(x, conv_w, fc1_w, fc1_b, fc2_w, fc2_b, out_w, out_b):
    dt = ml_dtypes.bfloat16
    f8 = ml_dtypes.float8_e4m3
    f32 = np.float32

    def q8(a):
        return np.clip(a, -192, 192).astype(f8)

    # Conv as a [784, 676] matrix (exact in fp64), folded into fc1.
    C = np.zeros((784, 676), dtype=np.float64)
    oy, ox = np.meshgrid(np.arange(26), np.arange(26), indexing="ij")
    cols = (oy * 26 + ox).ravel()
    for ky in range(3):
        for kx in range(3):
            rows = ((oy + ky) * 28 + (ox + kx)).ravel()
            np.add.at(C, (rows, cols), float(conv_w[ky, kx]))
    W1 = C @ fc1_w.T.astype(np.float64)  # [784, 512]

    # fp8 W1 (features 0..511), DoubleRow block layout [p, 2P+ko, m]
    w18 = np.zeros((128, 4, 512), dtype=f8)
    for kk in range(4):
        w18[:, kk, :] = q8(SW * W1[kk * 128:(kk + 1) * 128, :])
    # bf16 W1 (features 512..767), scaled to match
    w1 = np.zeros((128, 2 * 512), dtype=np.float64)
    for kb in range(2):
        w1[:, kb * 512:(kb + 1) * 512] = \
            SW * W1[512 + kb * 128:512 + (kb + 1) * 128, :]
    w1 = w1.astype(f32).astype(dt)

    w2 = np.ascontiguousarray(
        np.ascontiguousarray(fc2_w.T).reshape(KT2, 128, 512).transpose(1, 0, 2)
    ).reshape(128, KT2 * 512).astype(f32).astype(dt)
    b3col = np.zeros((128, 1), dtype=np.float64)
    b3col[:MO, 0] = out_b
    b = np.ascontiguousarray(
        np.concatenate(
            [fc1_b.reshape(MT1, 128).T, fc2_b.reshape(MT2, 128).T, b3col],
            axis=1,
        )
    ).astype(f32)
    # merged consts: scaled w1 k6 block | w3 k-tiles | 0/1 selection matrix
    cstm = np.zeros((128, 184), dtype=np.float64)
    for j in range(4):
        cstm[32 * j:32 * j + 16, 0:128] = \
            SW * W1[768:784, 128 * j:128 * (j + 1)]
    cstm[:, 128:168] = np.ascontiguousarray(out_w.T.astype(np.float64)).reshape(
        KT3, 128, MO).transpose(1, 0, 2).reshape(128, KT3 * MO)
    for j in range(MO):
        for q in range(KT3):
            cstm[32 * q + j, 168 + j] = 1.0
    cstm = cstm.astype(f32).astype(dt)

    in_maps = []
    for c in range(NCORES):
        xc = x[c * BC:(c + 1) * BC].T.astype(np.float64)  # [784, BC]
        x8 = np.zeros((NCH, 128, 4, CH), dtype=f8)
        x16 = np.zeros((NCH, 128, XW16), dtype=dt)
        for n in range(NCH):
            cn = slice(n * CH, (n + 1) * CH)
            for kk in range(4):
                x8[n, :, kk, :] = q8(SX * xc[kk * 128:(kk + 1) * 128, cn])
            for kb in range(2):
                x16[n, :, kb * CH:(kb + 1) * CH] = (
                    SX * xc[512 + kb * 128:512 + (kb + 1) * 128, cn]
                ).astype(f32)
            rep = (SX * xc[768:784, cn]).astype(f32)
            for j in range(4):
                x16[n, 32 * j:32 * j + 16, 2 * CH:] = rep
        in_maps.append(
            {"x8": x8, "x16": x16, "w18": w18, "w1": w1, "w2": w2,
             "cst": cstm, "b": b}
        )
    return in_maps


def kernel(x, conv_w, fc1_w, fc1_b, fc2_w, fc2_b, out_w, out_b, _results=None):
    from concourse.bass_utils import run_bass_kernel_spmd

    x, conv_w, fc1_w, fc1_b, fc2_w, fc2_b, out_w, out_b = (
        np.asarray(a)
        for a in (x, conv_w, fc1_w, fc1_b, fc2_w, fc2_b, out_w, out_b)
    )
    nc = _build()
    in_maps = _prep_inputs(x, conv_w, fc1_w, fc1_b, fc2_w, fc2_b, out_w, out_b)
    res = run_bass_kernel_spmd(nc, in_maps, core_ids=list(range(NCORES)))
    if _results is not None:
        _results.append(res)
    out = np.empty((B, 10), dtype=np.float32)
    for c in range(NCORES):
        out[c * BC:(c + 1) * BC, :] = res.results[c]["out"].T
    return out


# revision 16
# speedup vs baseline: 1.0312x; 1.0080x over previous
"""Trainium2 Bass kernel for nn_DigitConvolutionalModel (dense_cnn).

Math: the 3x3 valid conv is linear in x, so it folds into fc1:
    conv(x) @ fc1_w.T == x @ (C @ fc1_w.T)  with C [784, 676] the conv matrix.
The whole model is then a 3-layer MLP:
    out = relu(relu(x @ W1 + b1) @ W2 + b2) @ W3 + b3
with W1 = C @ fc1_w.T [784,512], W2 = fc2_w.T [512,512], W3 = out_w.T [512,10].

Sharding: pure data parallelism; batch 32768 -> 8 cores x 4096 rows.

On-chip formulation is fully transposed (features on SBUF partitions, batch on
the free dim): each layer computes h^T = act(W_l as lhsT, rhs = h_{l-1}^T).

PE-array scheduling (HW-measured model):
 - flags!=3 (accumulating) matmuls drain PSUM at ~259ns/512 cols; flags=3
   (start+stop) at ~216ns; LDWEIGHTS hides in the drain windows.
 - Layer 1 contraction (K=784): features 0..511 run in fp8-E4M3 DoubleRow
   (K=256/pass -> 2 passes instead of 4; measured rel err 1.81% < 2e-2).
   x scaled by 16, W1 by 128 (power-of-2, exact); the h1 activation rescales
   by 1/2048.  Features 512..767 (bf16) run FIRST so the wide DoubleRow
   LDWEIGHTS (225ns) hide behind bf16 drains.  The 16-row remainder is ONE
   pass of 4 concurrent row-tiled matmuls (tile_position=(32*mi,0), K=16).
 - Layer 3 (M=10): ONE concurrent pass of 4 col-tiled flags=3 matmuls
   (tile_position=(0,32*ki)) writes the k-partials to partition strips of a
   PSUM bank; a full-partition DVE scalar_tensor_tensor copy (bf16) + one
   matmul against a 0/1 selection matrix reduces the strips.  Two banks
   alternate by chunk parity so pack(n-2) and reduce(n-3) sit adjacent in
   the small-shape window and layer 2 runs uninterrupted.
   (tensor_copy would lower to CAST and pull an extra ucode library load
   into the preamble, +3us; stt avoids that.)
 - h2 relu+bias runs on the idle vector engine.
 - Warm-up: dummy N=128 matmuls + bridges keep the PE busy through the
   preamble/DMA fill so HAM reaches K=8/8 early.
"""

import numpy as np
import ml_dtypes

NCORES = 8
B = 32768
BC = B // NCORES  # rows per core
CH = 512          # batch chunk = matmul moving free dim = one fp32 PSUM bank
NCH = BC // CH
MT1 = 4           # 512 out feats = 4 m-tiles of 128
KT2, MT2 = 4, 4   # layer-2: K=512, M=512
KT3, MO = 4, 10   # layer-3: K=512, M=10
XW16 = 3 * CH     # bf16 x cols: k4 | k5 | replicated k6 block
SX = 16.0         # fp8 x scale (pow2, exact)
SW = 128.0        # fp8 W1 scale
SCALE = SX * SW
N_WARM = 36       # N=128 dummy matmuls bridging preamble -> first k-tile

_cache = {}


def _build():
    """Trace + compile the Bass program once per process."""
    if "nc" in _cache:
        return _cache["nc"]

    from contextlib import ExitStack

    import concourse.bass as bass
    import concourse.tile as tile
    from concourse import bacc, mybir
    from concourse.bass import ts, ds

    DT = mybir.dt.bfloat16
    F8 = mybir.dt.float8e4
    F32 = mybir.dt.float32
    DR = mybir.MatmulPerfMode.DoubleRow
    Relu = mybir.ActivationFunctionType.Relu
    Ident = mybir.ActivationFunctionType.Identity
    Add = mybir.AluOpType.add
    Max = mybir.AluOpType.max

    from concourse.vector_clock import ScopedClock

    class _FastExitTileContext(tile.TileContext):
        """Skip the exit semaphore-clear chain + second barrier (~2us tail)."""

        def _drain_and_barrier(self, tick_clock, wait_clock):
            drain_inst = self.nc.sync.drain()
            wait_clock.add_sem_waits(
                drain_inst.ins, ScopedClock({None: tick_clock.global_clock})
            )
            popped = self.nc._tile_sem_poison_stack.pop()
            assert popped is self._sem_poison

    nc = bacc.Bacc(
        "TRN2",
        target_bir_lowering=False,
        debug=False,
        enable_asserts=False,
        num_devices=NCORES,
        enable_partition_id=False,
    )

    # fp8 x: [chunk, partition, 2P+ko, col]; fp8 W1: [partition, 2P+ko, m]
    x8_d = nc.dram_tensor("x8", [NCH, 128, 4, CH], F8, kind="ExternalInput")
    x16_d = nc.dram_tensor("x16", [NCH, 128, XW16], DT, kind="ExternalInput")
    w18_d = nc.dram_tensor("w18", [128, 4, 512], F8, kind="ExternalInput")
    w1_d = nc.dram_tensor("w1", [128, 2 * 512], DT, kind="ExternalInput")
    w2_d = nc.dram_tensor("w2", [128, KT2 * 512], DT, kind="ExternalInput")
    # merged small consts, one DMA: w1 k6 block | w3 k-tiles | 0/1 sel matrix
    cst_d = nc.dram_tensor("cst", [128, 184], DT, kind="ExternalInput")
    b_d = nc.dram_tensor("b", [128, MT1 + MT2 + 1], F32, kind="ExternalInput")
    out_d = nc.dram_tensor("out", [MO, BC], F32, kind="ExternalOutput")

    with _FastExitTileContext(nc) as tc, ExitStack() as ctx:
        consts = ctx.enter_context(tc.tile_pool(name="consts", bufs=1))
        x8_pool = ctx.enter_context(tc.tile_pool(name="x8p", bufs=5))
        x16_pool = ctx.enter_context(tc.tile_pool(name="x16p", bufs=5))
        h1_pool = ctx.enter_context(tc.tile_pool(name="h1", bufs=8))
        h2_pool = ctx.enter_context(tc.tile_pool(name="h2", bufs=8))
        sb3_pool = ctx.enter_context(tc.tile_pool(name="sb3", bufs=2))
        oc_pool = ctx.enter_context(tc.tile_pool(name="oc", bufs=2))
        ps1 = ctx.enter_context(tc.tile_pool(name="ps1", bufs=4, space="PSUM"))
        ps2 = ctx.enter_context(tc.tile_pool(name="ps2", bufs=2, space="PSUM"))
        ps3 = ctx.enter_context(tc.tile_pool(name="ps3", bufs=2, space="PSUM"))

        # --- PE pre-warm from the earliest post-preamble instant ---
        warm_sb = consts.tile([128, 128], DT, name="warm_sb")
        nc.vector.memset(warm_sb[:], 0.0)
        zeros = consts.tile([128, CH], DT, name="zeros")
        nc.vector.memset(zeros[:], 0.0)
        warm_ps = ps2.tile([128, 128], F32, name="warm_ps", tag="ps2")
        for _ in range(N_WARM):
            nc.tensor.matmul(warm_ps[:], warm_sb[:], warm_sb[:],
                             start=True, stop=True)

        def bridge(k):
            for _ in range(k):
                nc.tensor.matmul(warm_ps[:], warm_sb[:], warm_sb[:],
                                 start=True, stop=True)

        # --- input DMAs ---
        # ring A = scalar engine queue, ring B = sync engine queue.
        x8_0 = consts.tile([128, 4, CH], F8, name="x8_0")
        nc.scalar.dma_start(x8_0[:, 0:2, :], x8_d[0][:, 0:2, :])
        w18_sb = consts.tile([128, 4, 512], F8, name="w18_sb")
        nc.sync.dma_start(w18_sb[:, 0:2, :], w18_d[:, 0:2, :])

        x16_0 = consts.tile([128, XW16], DT, name="x16_0")
        nc.scalar.dma_start(x16_0[:], x16_d[0])
        nc.sync.dma_start(x8_0[:, 2:4, :], x8_d[0][:, 2:4, :])
        nc.sync.dma_start(w18_sb[:, 2:4, :], w18_d[:, 2:4, :])

        w1_sb = consts.tile([128, 2 * 512], DT, name="w1_sb")
        nc.sync.dma_start(w1_sb[:], w1_d[:])
        b_sb = consts.tile([128, MT1 + MT2 + 1], F32, name="b_sb")
        nc.sync.dma_start(b_sb[:], b_d[:])

        x8_1 = consts.tile([128, 4, CH], F8, name="x8_1")
        nc.scalar.dma_start(x8_1[:], x8_d[1])

        cst = consts.tile([128, 184], DT, name="cst")
        nc.sync.dma_start(cst[:], cst_d[:])
        x16_1 = consts.tile([128, XW16], DT, name="x16_1")
        nc.sync.dma_start(x16_1[:], x16_d[1])

        w2_sb = consts.tile([128, KT2 * 512], DT, name="w2_sb")
        nc.scalar.dma_start(w2_sb[:], w2_d[:])

        x8_2 = consts.tile([128, 4, CH], F8, name="x8_2")
        nc.sync.dma_start(x8_2[:], x8_d[2])
        x16_2 = consts.tile([128, XW16], DT, name="x16_2")
        nc.scalar.dma_start(x16_2[:], x16_d[2])

        x8c = [x8_0, x8_1, x8_2] + [None] * (NCH - 3)
        x16c = [x16_0, x16_1, x16_2] + [None] * (NCH - 3)

        def fetch(n):
            t8 = x8_pool.tile([128, 4, CH], F8, name=f"x8c{n}", tag="x8")
            t16 = x16_pool.tile([128, XW16], DT, name=f"x16c{n}", tag="x16")
            if n % 2 == 1:
                nc.scalar.dma_start(t8[:], x8_d[n])
                nc.sync.dma_start(t16[:], x16_d[n])
            else:
                nc.sync.dma_start(t8[:], x8_d[n])
                nc.scalar.dma_start(t16[:], x16_d[n])
            x8c[n] = t8
            x16c[n] = t16

        # ---- pipeline stages ----
        h1s = {}   # n -> h1 tiles
        h2s = {}   # n -> h2 tiles
        pscs = {}  # n -> layer-3 psum tile
        sb3s = {}  # n -> bf16 strip-copy tile

        # two alternating layer-3 PSUM banks; strips at partitions 32k..+10.
        # memset once: stale NaN/Inf in the never-written rows would poison
        # the 0-weighted lanes of the selection reduce (0*NaN = NaN).
        ps3_t = [ps3.tile([128, CH], F32, name=f"ps3_{i}", tag="ps3")
                 for i in range(2)]
        nc.vector.memset(ps3_t[0][:], 0.0)
        nc.vector.memset(ps3_t[1][:], 0.0)

        def l1_dr(n, ps, P, start=False):
            for mi in range(MT1):
                nc.tensor.matmul(
                    ps[mi][:],
                    w18_sb[:, 2 * P:2 * P + 2, ds(mi * 128, 128)],
                    x8c[n][:, 2 * P:2 * P + 2, :],
                    start=start, stop=False, perf_mode=DR,
                )

        def l1_kbf(n, ps, kb, start=False):
            for mi in range(MT1):
                nc.tensor.matmul(
                    ps[mi][:],
                    w1_sb[:, ds(kb * 512 + mi * 128, 128)],
                    x16c[n][:, ts(kb, CH)],
                    start=start, stop=False,
                )

        def l1_k6(n, ps):
            # 16-row remainder: 4 concurrent row-tiled matmuls, one pass
            for mi in range(MT1):
                nc.tensor.matmul(
                    ps[mi][:],
                    cst[32 * mi:32 * mi + 16, 0:128],
                    x16c[n][32 * mi:32 * mi + 16, ds(2 * CH, CH)],
                    start=False, stop=True,
                    tile_position=(32 * mi, 0),
                )

        def l1_acts(n, ps):
            h1t = [
                h1_pool.tile([128, CH], DT, name=f"h1_{n}_{mi}", tag="h1")
                for mi in range(MT1)
            ]
            for mi in range(MT1):
                nc.scalar.activation(
                    h1t[mi][:], ps[mi][:], Relu, bias=b_sb[:, mi:mi + 1],
                    scale=1.0 / SCALE,
                )
            h1s[n] = h1t

        def l3_pack(n):
            # ONE concurrent pass: 4 col-tiled flags=3 matmuls write the 4
            # k-partials of chunk n to partition strips of bank n%2
            psc = ps3_t[n % 2]
            pscs[n] = psc
            h2t = h2s.pop(n)
            for ki in range(KT3):
                nc.tensor.matmul(
                    psc[32 * ki:32 * ki + MO, :],
                    cst[:, ds(128 + ki * MO, MO)], h2t[ki][:],
                    start=True, stop=True,
                    tile_position=(0, 32 * ki),
                )

        def l3_copy(n):
            # full-partition DVE stt copies the strips to bf16 SBUF; emitted
            # after layer2 so it never delays the h2 stt chain (ps2 has only
            # 2 rotating banks)
            sb = sb3_pool.tile([128, CH], DT, name=f"sb3_{n}", tag="sb3")
            nc.vector.scalar_tensor_tensor(
                sb[:], pscs[n][:], zeros[:, 0:1], zeros[:], Add, Add,
            )
            sb3s[n] = sb

        def l3_fin(n):
            # reduce the 4 strips with a 0/1 selection matmul (contracts
            # over partitions), then bias + store
            psc = pscs.pop(n)
            nc.tensor.matmul(psc[:MO, :], cst[:, 168:168 + MO],
                             sb3s.pop(n)[:], start=True, stop=True)
            oct_ = oc_pool.tile([MO, CH], F32, name=f"oc_{n}", tag="oc")
            nc.scalar.activation(
                oct_[:], psc[:MO, :], Ident,
                bias=b_sb[:MO, MT1 + MT2:MT1 + MT2 + 1],
            )
            nc.sync.dma_start(out_d[:, ts(n, CH)], oct_[:],
                              single_packet=True)

        def layer2(n):
            h1t = h1s.pop(n)
            h2t = [None] * MT2
            seq_ps = None
            if n == NCH - 1:
                # tail chunk: layer 3 as 4 sequential accumulating matmuls
                # riding between the L2 mi-groups as each h2 ACT lands -- no
                # pack/copy/reduce chain left on the critical tail.  ps1
                # banks are idle in the epilogue.
                seq_ps = ps1.tile([128, CH], F32, name="ps3_seq", tag="ps1")
                pscs[n] = seq_ps
            for mi in range(MT2):
                ps = ps2.tile([128, CH], F32, name=f"ps2_{n}_{mi}", tag="ps2")
                for ki in range(KT2):
                    nc.tensor.matmul(
                        ps[:],
                        w2_sb[:, ds(ki * 512 + mi * 128, 128)],
                        h1t[ki][:],
                        start=(ki == 0), stop=(ki == KT2 - 1),
                    )
                h2t[mi] = h2_pool.tile([128, CH], DT, name=f"h2_{n}_{mi}",
                                       tag="h2")
                if n == NCH - 1:
                    # relu on the (then idle) scalar engine, then the L3
                    # k-partial matmul for this mi immediately
                    nc.scalar.activation(
                        h2t[mi][:], ps[:], Relu,
                        bias=b_sb[:, MT1 + mi:MT1 + mi + 1],
                    )
                    nc.tensor.matmul(
                        seq_ps[:MO, :], cst[:, ds(128 + mi * MO, MO)],
                        h2t[mi][:],
                        start=(mi == 0), stop=(mi == MT2 - 1),
                    )
                else:
                    # relu+bias on the vector engine: max(ps + b, 0)
                    nc.vector.scalar_tensor_tensor(
                        h2t[mi][:], ps[:], b_sb[:, MT1 + mi:MT1 + mi + 1],
                        zeros[:], Add, Max,
                    )
            h2s[n] = h2t

        def l3seq_fin(n):
            # tail chunk: store via the scalar ring -- the sync engine is
            # busy issuing chunk-6's store and then runs the exit sequence;
            # splitting the two final DMAs across rings shortens the tail
            psc = pscs.pop(n)
            oct_ = oc_pool.tile([MO, CH], F32, name=f"oc_{n}", tag="oc")
            nc.scalar.activation(
                oct_[:], psc[:MO, :], Ident,
                bias=b_sb[:MO, MT1 + MT2:MT1 + MT2 + 1],
            )
            nc.scalar.dma_start(out_d[:, ts(n, CH)], oct_[:],
                                single_packet=True)

        # ---- software pipeline ----
        # iter n: layer1(n); chunk n-2's layer-3 pack and chunk n-3's reduce
        # sit adjacent in the small-shape window after the k6 pass; layer2
        # of chunk n-1 then runs uninterrupted.
        for n in range(NCH):
            if 1 <= n <= NCH - 3:
                fetch(n + 2)
            ps = [
                ps1.tile([128, CH], F32, name=f"ps1_{n}_{mi}", tag="ps1")
                for mi in range(MT1)
            ]
            if n == 0:
                # chunk 0 follows DMA-arrival order: fp8 pieces land first
                bridge(8)
                l1_dr(n, ps, 0, start=True)
                bridge(12)
                l1_dr(n, ps, 1)
                bridge(6)
                l1_kbf(n, ps, 0)
                bridge(4)
                l1_kbf(n, ps, 1)
                bridge(3)
            else:
                # bf16 first: the wide DoubleRow LDWEIGHTS then hide behind
                # bf16 drain windows
                l1_kbf(n, ps, 0, start=True)
                l1_kbf(n, ps, 1)
                if n == 1:
                    # chunk-1 fp8 x is still in flight on the DMA ring;
                    # keep HAM busy so it doesn't re-throttle
                    bridge(8)
                l1_dr(n, ps, 0)
                if n == 1:
                    bridge(4)
                l1_dr(n, ps, 1)
            l1_k6(n, ps)
            if n >= 2:
                l3_pack(n - 2)
            if n >= 3:
                l3_fin(n - 3)
            l1_acts(n, ps)
            if n == 1:
                # cover the first h1-activation latency before layer2(0)
                bridge(8)
            if n >= 1:
                layer2(n - 1)
            if n >= 2:
                l3_copy(n - 2)

        # epilogue: chunk-5/6 L3 work overlaps layer2(7); chunk 7 finishes
        # through the sequential path with no copy/reduce on the tail
        l3_pack(NCH - 2)
        l3_copy(NCH - 2)
        l3_fin(NCH - 3)
        layer2(NCH - 1)
        l3_fin(NCH - 2)
        l3seq_fin(NCH - 1)

    nc.compile()
    _cache["nc"] = nc
    return nc


def _prep_inputs(x, conv_w, fc1_w, fc1_b, fc2_w, fc2_b, out_w, out_b):
    dt = ml_dtypes.bfloat16
    f8 = ml_dtypes.float8_e4m3
    f32 = np.float32

    def q8(a):
        return np.clip(a, -192, 192).astype(f8)

    # Conv as a [784, 676] matrix (exact in fp64), folded into fc1.
    C = np.zeros((784, 676), dtype=np.float64)
    oy, ox = np.meshgrid(np.arange(26), np.arange(26), indexing="ij")
    cols = (oy * 26 + ox).ravel()
    for ky in range(3):
        for kx in range(3):
            rows = ((oy + ky) * 28 + (ox + kx)).ravel()
            np.add.at(C, (rows, cols), float(conv_w[ky, kx]))
    W1 = C @ fc1_w.T.astype(np.float64)  # [784, 512]

    # fp8 W1 (features 0..511), DoubleRow block layout [p, 2P+ko, m]
    w18 = np.zeros((128, 4, 512), dtype=f8)
    for kk in range(4):
        w18[:, kk, :] = q8(SW * W1[kk * 128:(kk + 1) * 128, :])
    # bf16 W1 (features 512..767), scaled to match
    w1 = np.zeros((128, 2 * 512), dtype=np.float64)
    for kb in range(2):
        w1[:, kb * 512:(kb + 1) * 512] = \
            SW * W1[512 + kb * 128:512 + (kb + 1) * 128, :]
    w1 = w1.astype(f32).astype(dt)

    w2 = np.ascontiguousarray(
        np.ascontiguousarray(fc2_w.T).reshape(KT2, 128, 512).transpose(1, 0, 2)
    ).reshape(128, KT2 * 512).astype(f32).astype(dt)
    b3col = np.zeros((128, 1), dtype=np.float64)
    b3col[:MO, 0] = out_b
    b = np.ascontiguousarray(
        np.concatenate(
            [fc1_b.reshape(MT1, 128).T, fc2_b.reshape(MT2, 128).T, b3col],
            axis=1,
        )
    ).astype(f32)
    # merged consts: scaled w1 k6 block | w3 k-tiles | 0/1 selection matrix
    cstm = np.zeros((128, 184), dtype=np.float64)
    for j in range(4):
        cstm[32 * j:32 * j + 16, 0:128] = \
            SW * W1[768:784, 128 * j:128 * (j + 1)]
    cstm[:, 128:168] = np.ascontiguousarray(out_w.T.astype(np.float64)).reshape(
        KT3, 128, MO).transpose(1, 0, 2).reshape(128, KT3 * MO)
    for j in range(MO):
        for q in range(KT3):
            cstm[32 * q + j, 168 + j] = 1.0
    cstm = cstm.astype(f32).astype(dt)

    in_maps = []
    for c in range(NCORES):
        xc = x[c * BC:(c + 1) * BC].T.astype(np.float64)  # [784, BC]
        x8 = np.zeros((NCH, 128, 4, CH), dtype=f8)
        x16 = np.zeros((NCH, 128, XW16), dtype=dt)
        for n in range(NCH):
            cn = slice(n * CH, (n + 1) * CH)
            for kk in range(4):
                x8[n, :, kk, :] = q8(SX * xc[kk * 128:(kk + 1) * 128, cn])
            for kb in range(2):
                x16[n, :, kb * CH:(kb + 1) * CH] = (
                    SX * xc[512 + kb * 128:512 + (kb + 1) * 128, cn]
                ).astype(f32)
            rep = (SX * xc[768:784, cn]).astype(f32)
            for j in range(4):
                x16[n, 32 * j:32 * j + 16, 2 * CH:] = rep
        in_maps.append(
            {"x8": x8, "x16": x16, "w18": w18, "w1": w1, "w2": w2,
             "cst": cstm, "b": b}
        )
    return in_maps


def kernel(x, conv_w, fc1_w, fc1_b, fc2_w, fc2_b, out_w, out_b, _results=None):
    from concourse.bass_utils import run_bass_kernel_spmd

    x, conv_w, fc1_w, fc1_b, fc2_w, fc2_b, out_w, out_b = (
        np.asarray(a)
        for a in (x, conv_w, fc1_w, fc1_b, fc2_w, fc2_b, out_w, out_b)
    )
    nc = _build()
    in_maps = _prep_inputs(x, conv_w, fc1_w, fc1_b, fc2_w, fc2_b, out_w, out_b)
    res = run_bass_kernel_spmd(nc, in_maps, core_ids=list(range(NCORES)))
    if _results is not None:
        _results.append(res)
    out = np.empty((B, 10), dtype=np.float32)
    for c in range(NCORES):
        out[c * BC:(c + 1) * BC, :] = res.results[c]["out"].T
    return out
